# revision 1
# baseline (speedup 1.0000x reference)
"""DeepFM forward kernel for 8 Trainium2 NeuronCores (Bass/Tile).

Math (per batch row b):
    lin[b] = x[b] @ w + b0
    C[b]   = sum_k (x[b] @ v)_k^2
    Bq[b]  = sum_f s[f] * x[b,f]^2,   s[f] = sum_k v[f,k]^2
    out[b] = sigmoid(lin[b] + 0.5*C[b] - 0.5*Bq[b])

Data-parallel: batch 16384 sharded 8 ways (2048 rows/core); parameters
replicated. x is shipped pre-transposed (features on partitions) so every
matmul contracts over the partition dim with no on-chip transposes.

Precision scheme (hardware fp32r truncates matmul inputs to 11 mantissa
bits; engine writes to f32r tiles round to the same grid):
  - A-stream (xv + lin): 3 fp32r passes  x11@vw11 + x11@vwl + xl@vw11
    where x11 = round11(x), xl = x - x11 (exact), vw split likewise.
    Residual ~2^-22 relative — fp32-level.
  - B-stream (PRECISE_B): 2 fp32r passes over m = s*x^2 (ACT Square with
    per-feature sqrt(s) scale): hi = round11(m) and the exact residual
    m - hi, accumulated into the same PSUM row. End-to-end output error is
    at the fp32 reference's own noise floor (~1e-6 norm rel).
    With PRECISE_B=False: single truncated pass, ~2e-4 absmax, ~15% faster.
"""

import numpy as np

import concourse.bass as bass
import concourse.tile as tile
from concourse import bacc, mybir
from concourse.bass_utils import run_bass_kernel_spmd

BATCH, FIELD, EMBED = 16384, 2048, 64
NCORES = 8
BS = BATCH // NCORES   # 2048 batch rows per core
NCHUNK = 512           # psum free-dim per matmul
KTILES = FIELD // 128  # 16 contraction tiles
NCHUNKS = BS // NCHUNK  # 4 batch chunks per core
M = EMBED + 1          # 65 stationary columns: v plus w

F32 = mybir.dt.float32
F32R = mybir.dt.float32r
AF = mybir.ActivationFunctionType

# Two-pass B-stream: adds an exact-residual pass for the quadratic term,
# taking the output to fp32-reference accuracy (~1e-7) at ~10% more time.
PRECISE_B = True


def _build_nc():
    nc = bacc.Bacc("TRN2", target_bir_lowering=False, debug=False)

    xt = nc.declare_dram_parameter("xt", [FIELD, BS], F32, isOutput=False)
    # host-packed SBUF images: [128, KTILES*M], [128, KTILES]
    vw11i = nc.declare_dram_parameter("vw11i", [128, KTILES * M], F32R, isOutput=False)
    vwli = nc.declare_dram_parameter("vwli", [128, KTILES * M], F32R, isOutput=False)
    sqsi = nc.declare_dram_parameter("sqsi", [128, KTILES], F32, isOutput=False)
    red = nc.declare_dram_parameter("red", [97, 1], F32, isOutput=False)
    ones = nc.declare_dram_parameter("ones", [128, 1], F32R, isOutput=False)
    bvec = nc.declare_dram_parameter("bvec", [1, 1], F32, isOutput=False)
    y = nc.declare_dram_parameter("y", [NCHUNKS, NCHUNK], F32, isOutput=True)

    with tile.TileContext(nc) as tc:
        with (
            tc.tile_pool(name="consts", bufs=1) as consts,
            tc.tile_pool(name="xin", bufs=5) as xin,
            tc.tile_pool(name="x11p", bufs=5) as x11p,
            tc.tile_pool(name="xlp", bufs=4) as xlp,
            tc.tile_pool(name="mfp", bufs=3) as mfp,
            tc.tile_pool(name="mrp", bufs=3) as mrp,
            tc.tile_pool(name="mlp", bufs=3) as mlp,
            tc.tile_pool(name="redrhs", bufs=4) as redrhs,
            tc.tile_pool(name="outp", bufs=2) as outp,
            tc.tile_pool(name="psA", bufs=NCHUNKS, space="PSUM") as psA,
            tc.tile_pool(name="psB", bufs=NCHUNKS, space="PSUM") as psB,
        ):
            # ---- replicated parameters, loaded once. All consts ride the
            # ACT queue so SP streams x and Pool starts x11 copies at t=0;
            # the ones DMA is issued after the first stripe (see below) so it
            # doesn't block Pool's first x11 copy. ----
            vw11 = consts.tile([128, KTILES * M], F32R)
            nc.gpsimd.dma_start(vw11[:, :], vw11i[:, :])
            sqs_sb = consts.tile([128, KTILES], F32)
            nc.scalar.dma_start(sqs_sb[:, :], sqsi[:, :])
            ones_sb = consts.tile([128, 1], F32R)
            nc.gpsimd.dma_start(ones_sb[:, :], ones[:, :])
            vwl = consts.tile([128, KTILES * M], F32R)
            nc.scalar.dma_start(vwl[:, :], vwli[:, :])
            red_sb = consts.tile([97, 1], F32)
            nc.scalar.dma_start(red_sb[:, :], red[:, :])
            b_sb = consts.tile([1, 1], F32)
            nc.scalar.dma_start(b_sb[:, :], bvec[:, :])

            psumA = [
                psA.tile([M, NCHUNK], F32, name=f"psumA{n}", tag="psumA")
                for n in range(NCHUNKS)
            ]
            psumB = [
                psB.tile([1, NCHUNK], F32, name=f"psumB{n}", tag="psumB")
                for n in range(NCHUNKS)
            ]

            def process(k, pieces):
                """One contraction stripe k, split into `pieces` column blocks
                (list of (col_lo, col_hi)); each block covers whole chunks."""
                vw11_k = vw11[:, k * M:(k + 1) * M]
                vwl_k = vwl[:, k * M:(k + 1) * M]
                first, last = k == 0, k == KTILES - 1
                for lo, hi in pieces:
                    w = hi - lo
                    xk = xin.tile([128, w], F32, name=f"xk{k}_{lo}", tag="xk")
                    nc.sync.dma_start(xk[:, :], xt[k * 128:(k + 1) * 128, lo:hi])
                    # Engine balance: DVE is the busiest engine (the two
                    # full-rate f32 subs); hand a 128-col slice of each sub
                    # to GPSIMD, which has slack.
                    spl = w - 256 if w >= 1024 else w
                    x11 = x11p.tile([128, w], F32R, name=f"x11{k}_{lo}", tag="x11")
                    nc.gpsimd.tensor_copy(x11[:, :], xk[:, :])
                    xl = xlp.tile([128, w], F32R, name=f"xl{k}_{lo}", tag="xl")
                    nc.vector.tensor_sub(xl[:, :spl], xk[:, :spl], x11[:, :spl])
                    if spl < w:
                        nc.gpsimd.tensor_sub(
                            xl[:, spl:], xk[:, spl:], x11[:, spl:]
                        )
                    if PRECISE_B:
                        # m = s*x^2 in f32; hi-part = round11(m) on Pool;
                        # lo-part = m - hi (exact) on DVE. Both pass the PE
                        # untruncated.
                        mf = mfp.tile([128, w], F32, name=f"mf{k}_{lo}", tag="mf")
                        nc.scalar.activation(
                            mf[:, :], xk[:, :], AF.Square, scale=sqs_sb[:, k:k + 1]
                        )
                        mr = mrp.tile([128, w], F32R, name=f"mr{k}_{lo}", tag="mr")
                        nc.gpsimd.tensor_copy(mr[:, :], mf[:, :])
                        ml = mlp.tile([128, w], F32R, name=f"ml{k}_{lo}", tag="ml")
                        nc.vector.tensor_sub(ml[:, :spl], mf[:, :spl], mr[:, :spl])
                        if spl < w:
                            nc.gpsimd.tensor_sub(
                                ml[:, spl:], mf[:, spl:], mr[:, spl:]
                            )
                    else:
                        mr = mrp.tile([128, w], F32R, name=f"mr{k}_{lo}", tag="mr")
                        nc.scalar.activation(
                            mr[:, :], xk[:, :], AF.Square, scale=sqs_sb[:, k:k + 1]
                        )
                        ml = None

                    chunks = range(lo // NCHUNK, hi // NCHUNK)
                    # x11-dependent matmuls first (ready earliest), then xl/m
                    for n in chunks:
                        sl = slice(n * NCHUNK - lo, (n + 1) * NCHUNK - lo)
                        nc.tensor.matmul(
                            psumA[n][:, :], vw11_k, x11[:, sl],
                            start=first, stop=False,
                        )
                        nc.tensor.matmul(
                            psumA[n][:, :], vwl_k, x11[:, sl],
                            start=False, stop=False,
                        )
                    for n in chunks:
                        sl = slice(n * NCHUNK - lo, (n + 1) * NCHUNK - lo)
                        nc.tensor.matmul(
                            psumA[n][:, :], vw11_k, xl[:, sl],
                            start=False, stop=last,
                        )
                    for n in chunks:
                        sl = slice(n * NCHUNK - lo, (n + 1) * NCHUNK - lo)
                        nc.tensor.matmul(
                            psumB[n][:, :], ones_sb[:, :], mr[:, sl],
                            start=first, stop=(last and not PRECISE_B),
                        )
                    if PRECISE_B:
                        for n in chunks:
                            sl = slice(n * NCHUNK - lo, (n + 1) * NCHUNK - lo)
                            nc.tensor.matmul(
                                psumB[n][:, :], ones_sb[:, :], ml[:, sl],
                                start=False, stop=last,
                            )

            # First and last stripes in quarters: the first fills the pipeline
            # quickly; the last lets each chunk close its accumulation (and
            # start its epilogue) without waiting for the whole-stripe subs.
            quarters = [(i * NCHUNK, (i + 1) * NCHUNK) for i in range(NCHUNKS)]
            process(0, quarters)
            for k in range(1, KTILES - 1):
                process(k, [(0, BS)])
            process(KTILES - 1, quarters)

            # ---- epilogue: batch same-function ACT ops to avoid table reloads ----
            rhss, psumCs = [], []
            for n in range(NCHUNKS):
                # rows 0..63 = (xv)^2, 64 = lin, 65..95 zero, 96 = Bq
                rhs = redrhs.tile([97, NCHUNK], F32, name=f"rhs{n}", tag="rhs")
                nc.scalar.activation(rhs[0:EMBED, :], psumA[n][0:EMBED, :], AF.Square)
                nc.gpsimd.memset(rhs[64:96, :], 0.0)
                rhss.append(rhs)
            for n in range(NCHUNKS):
                nc.vector.tensor_copy(rhss[n][64:65, :], psumA[n][EMBED:M, :])
                nc.vector.tensor_copy(rhss[n][96:97, :], psumB[n][:, :])
            for n in range(NCHUNKS):
                # reuse a freed psumA slot (all psumA released after rhs built)
                psumC = psA.tile([1, NCHUNK], F32, name=f"psumC{n}", tag="psumA")
                nc.tensor.matmul(
                    psumC[:, :], red_sb[:, :], rhss[n][:, :], start=True, stop=True
                )
                out_sb = outp.tile([1, NCHUNK], F32, name=f"out{n}", tag="out")
                nc.scalar.activation(
                    out_sb[:, :], psumC[:, :], AF.Sigmoid, bias=b_sb[0:1, 0:1]
                )
                nc.gpsimd.dma_start(y[n:n + 1, :], out_sb[:, :])

    nc.compile()
    return nc


_NC_CACHE = None


def _prep_inputs(x, w, b, v):
    x = np.ascontiguousarray(x, dtype=np.float32)
    w = np.asarray(w, dtype=np.float32).reshape(FIELD, 1)
    v = np.asarray(v, dtype=np.float32)
    b0 = float(np.asarray(b, dtype=np.float32).reshape(-1)[0])

    s64 = (v.astype(np.float64) ** 2).sum(axis=1)
    sqs = np.sqrt(s64).astype(np.float32)
    vw = np.concatenate([v, w], axis=1).astype(np.float32)  # [FIELD, M]

    # hi/lo split on the f32r (11-mantissa-bit) grid; vw11 + vwl == vw to
    # within half an f32 ulp, both pieces pass through the PE unaltered.
    ui = vw.view(np.uint32).astype(np.uint64)
    r = (((ui + (1 << 11)) >> 12) << 12) & 0xFFFFFFFF
    vw11 = r.astype(np.uint32).view(np.float32)
    ui_l = ((vw.astype(np.float64) - vw11).astype(np.float32)
            .view(np.uint32).astype(np.uint64))
    r_l = (((ui_l + (1 << 11)) >> 12) << 12) & 0xFFFFFFFF
    vwl = r_l.astype(np.uint32).view(np.float32)

    def pack(a):  # [FIELD, M] -> [128, KTILES*M] SBUF image
        return np.ascontiguousarray(
            a.reshape(KTILES, 128, M).transpose(1, 0, 2).reshape(128, KTILES * M)
        )

    vw11i, vwli = pack(vw11), pack(vwl)
    sqsi = np.ascontiguousarray(sqs.reshape(KTILES, 128).T)

    red = np.zeros((97, 1), np.float32)
    red[0:EMBED, 0] = 0.5
    red[EMBED, 0] = 1.0
    red[96, 0] = -0.5
    ones = np.ones((128, 1), np.float32)
    bvec = np.full((1, 1), b0, np.float32)

    in_maps = []
    for c in range(NCORES):
        xt_c = np.ascontiguousarray(x[c * BS:(c + 1) * BS, :].T)
        in_maps.append({
            "xt": xt_c, "vw11i": vw11i, "vwli": vwli, "sqsi": sqsi,
            "red": red, "ones": ones, "bvec": bvec,
        })
    return in_maps


def _run(x, w, b, v, **spmd_kwargs):
    global _NC_CACHE
    if _NC_CACHE is None:
        _NC_CACHE = _build_nc()
    nc = _NC_CACHE

    in_maps = _prep_inputs(x, w, b, v)
    res = run_bass_kernel_spmd(nc, in_maps, list(range(NCORES)), **spmd_kwargs)
    out = np.concatenate(
        [res.results[c]["y"].reshape(BS) for c in range(NCORES)]
    )
    return out.reshape(BATCH, 1).astype(np.float32), res


def kernel(x, w, b, v):
    out, _ = _run(x, w, b, v)
    return out



# revision 2
# speedup vs baseline: 2.1706x; 2.1706x over previous
"""DeepFM forward kernel for 8 Trainium2 NeuronCores (Bass/Tile).

Math (per batch row b):
    lin[b] = x[b] @ w
    C[b]   = sum_k (x[b] @ v)_k^2
    B[b]   = sum_f s[f] * x[b,f]^2,   s[f] = sum_k v[f,k]^2
    out[b] = sigmoid(lin[b] + b0 + 0.5*C[b] - 0.5*B[b])

Data-parallel: batch 16384 sharded 8 ways (2048 rows/core); parameters
replicated.

Key reformulation: ship u = x * sqrt(s) (per-feature scale folded on host)
in fp16, with v' = v/sqrt(s), w' = w/sqrt(s) as the stationary matrix.
Then xv = u @ v', lin = u @ w', and B = sum_f u_f^2 — the only on-chip
elementwise op is an unscaled square. fp16 halves HBM traffic (the
bottleneck) and runs the PE at full rate.

Engine budget per core (cost model):
  DMA  ~26us  <- bottleneck: 16 fp16 stripes [128,2048] @ ~1.58us + params
  PE   ~24us  A-pass (64 mm) + B for 9 stripes (ones-matmul) + epilogue
  ACT  ~16us  6 stripe squares + epilogue Square/Sigmoid
  DVE  ~13us  4 stripe squares + 5 tree-adds + lin copies
  GPS  ~10us  6 stripe squares

B-hybrid: 9 stripes' u^2 go straight to PSUM via ones-matmuls (keeps the
PE saturated so it stays at the 2.4GHz p-state); the other 7 accumulate
in fp16 via adds into two accumulators, folded in with 8 final matmuls.
"""

import numpy as np

import concourse.bass as bass
import concourse.tile as tile
from concourse import bacc, mybir
from concourse.bass_utils import run_bass_kernel_spmd

BATCH, FIELD, EMBED = 16384, 2048, 64
NCORES = 8
BS = BATCH // NCORES   # 2048 batch rows per core
NCHUNK = 512           # psum free-dim per matmul
KTILES = FIELD // 128  # 16 contraction stripes
NCHUNKS = BS // NCHUNK  # 4 batch chunks per core
M = EMBED + 1          # 65 stationary columns: v' plus w'

F32 = mybir.dt.float32
F16 = mybir.dt.float16
AF = mybir.ActivationFunctionType

# Stripes whose u^2 is summed on the PE (ones-matmul into psumB).
PE_B = {0, 2, 4, 6, 8, 10, 12, 14, 15}
# Tree stripes accumulate u^2 into two fp16 accumulators on DVE.
TREE_A = [1, 3, 5, 7]   # acc_a: sq1 written in place, then += sq3, sq5, sq7
TREE_B = [9, 11, 13]    # acc_b
# Engine for each stripe's square: v=DVE, g=GPS(pool), a=ACT
SQ_ENG = {0: "v", 1: "a", 2: "g", 3: "a", 4: "g", 5: "v", 6: "a", 7: "a",
          8: "g", 9: "v", 10: "a", 11: "g", 12: "a", 13: "v", 14: "g", 15: "g"}
QUARTERED = {0, 15}     # stripes whose DMA is split into 4 chunk loads


def _build_nc():
    nc = bacc.Bacc("TRN2", target_bir_lowering=False, debug=False)

    ut = nc.declare_dram_parameter("ut", [FIELD, BS], F16, isOutput=False)
    vwi = nc.declare_dram_parameter("vwi", [128, KTILES * M], F16, isOutput=False)
    red = nc.declare_dram_parameter("red", [M, 1], F16, isOutput=False)
    onesn = nc.declare_dram_parameter("onesn", [128, 1], F16, isOutput=False)
    bvec = nc.declare_dram_parameter("bvec", [1, 1], F32, isOutput=False)
    y = nc.declare_dram_parameter("y", [NCHUNKS, NCHUNK], F32, isOutput=True)

    with tile.TileContext(nc) as tc:
        with (
            tc.tile_pool(name="consts", bufs=1) as consts,
            tc.tile_pool(name="uin", bufs=4) as uin,
            tc.tile_pool(name="uq", bufs=1) as uq,
            tc.tile_pool(name="sqp", bufs=4) as sqp,
            tc.tile_pool(name="accs", bufs=1) as accs,
            tc.tile_pool(name="redrhs", bufs=4) as redrhs,
            tc.tile_pool(name="outp", bufs=2) as outp,
            tc.tile_pool(name="psA", bufs=NCHUNKS, space="PSUM") as psA,
            tc.tile_pool(name="psB", bufs=NCHUNKS, space="PSUM") as psB,
        ):
            # Replicated parameters, loaded once on non-SP queues so the SP
            # queue starts streaming u immediately.
            vw = consts.tile([128, KTILES * M], F16)
            nc.gpsimd.dma_start(vw[:, :], vwi[:, :])
            onesn_sb = consts.tile([128, 1], F16)
            nc.scalar.dma_start(onesn_sb[:, :], onesn[:, :])
            red_sb = consts.tile([M, 1], F16)
            nc.scalar.dma_start(red_sb[:, :], red[:, :])
            b_sb = consts.tile([1, 1], F32)
            nc.scalar.dma_start(b_sb[:, :], bvec[:, :])

            psumA = [
                psA.tile([M, NCHUNK], F32, name=f"psumA{n}", tag="psumA")
                for n in range(NCHUNKS)
            ]
            psumB = [
                psB.tile([1, NCHUNK], F32, name=f"psumB{n}", tag="psumB")
                for n in range(NCHUNKS)
            ]

            acc_a = accs.tile([128, BS], F16, name="acc_a")
            acc_b = accs.tile([128, BS], F16, name="acc_b")

            def sq_op(eng, dst, src):
                if eng == "v":
                    nc.vector.tensor_mul(dst, src, src)
                elif eng == "g":
                    nc.gpsimd.tensor_mul(dst, src, src)
                else:
                    nc.scalar.activation(dst, src, AF.Square)

            first_b = [True] * NCHUNKS  # psumB group start tracking

            def process(k):
                vw_k = vw[:, k * M:(k + 1) * M]
                firstA, lastA = k == 0, k == KTILES - 1
                if k in QUARTERED:
                    parts = [(n * NCHUNK, (n + 1) * NCHUNK) for n in range(NCHUNKS)]
                    u_k = uq.tile([128, BS], F16, name=f"uqt{k}", tag=f"uq{k}")
                    for lo, hi in parts:
                        nc.sync.dma_start(
                            u_k[:, lo:hi], ut[k * 128:(k + 1) * 128, lo:hi]
                        )
                else:
                    parts = [(0, BS)]
                    u_k = uin.tile([128, BS], F16, name=f"u{k}", tag="u")
                    nc.sync.dma_start(u_k[:, :], ut[k * 128:(k + 1) * 128, :])

                if k == TREE_A[0]:
                    sq_k = acc_a
                elif k == TREE_B[0]:
                    sq_k = acc_b
                else:
                    sq_k = sqp.tile([128, BS], F16, name=f"sq{k}", tag="sq")

                for lo, hi in parts:
                    for n in range(lo // NCHUNK, hi // NCHUNK):
                        sl = slice(n * NCHUNK, (n + 1) * NCHUNK)
                        nc.tensor.matmul(
                            psumA[n][:, :], vw_k, u_k[:, sl],
                            start=firstA, stop=lastA,
                        )
                    sq_op(SQ_ENG[k], sq_k[:, lo:hi], u_k[:, lo:hi])
                    if k in PE_B:
                        for n in range(lo // NCHUNK, hi // NCHUNK):
                            sl = slice(n * NCHUNK, (n + 1) * NCHUNK)
                            nc.tensor.matmul(
                                psumB[n][:, :], onesn_sb[:, :], sq_k[:, sl],
                                start=first_b[n], stop=False,
                            )
                            first_b[n] = False
                if k in TREE_A[1:]:
                    nc.vector.tensor_add(acc_a[:, :], acc_a[:, :], sq_k[:, :])
                elif k in TREE_B[1:]:
                    nc.vector.tensor_add(acc_b[:, :], acc_b[:, :], sq_k[:, :])

            for k in range(KTILES - 1):
                process(k)

            # Fold the two tree accumulators into psumB before the last stripe.
            for acc in (acc_a, acc_b):
                for n in range(NCHUNKS):
                    sl = slice(n * NCHUNK, (n + 1) * NCHUNK)
                    nc.tensor.matmul(
                        psumB[n][:, :], onesn_sb[:, :], acc[:, sl],
                        start=False, stop=False,
                    )

            process(KTILES - 1)

            # Epilogue per chunk: rhs rows 0..63 = (xv)^2 (fp16), row 64 = lin;
            # z lands in psumB via the final accumulating matmul.
            for n in range(NCHUNKS):
                rhs = redrhs.tile([M, NCHUNK], F16, name=f"rhs{n}", tag="rhs")
                nc.scalar.activation(rhs[0:EMBED, :], psumA[n][0:EMBED, :], AF.Square)
                nc.vector.tensor_copy(rhs[EMBED:M, :], psumA[n][EMBED:M, :])
                nc.tensor.matmul(
                    psumB[n][:, :], red_sb[:, :], rhs[:, :], start=False, stop=True
                )
                out_sb = outp.tile([1, NCHUNK], F32, name=f"out{n}", tag="out")
                nc.scalar.activation(
                    out_sb[:, :], psumB[n][:, :], AF.Sigmoid, bias=b_sb[0:1, 0:1]
                )
                nc.sync.dma_start(y[n:n + 1, :], out_sb[:, :])

    nc.compile()
    return nc


_NC_CACHE = None


def _prep_inputs(x, w, b, v):
    x = np.asarray(x, dtype=np.float32)
    w = np.asarray(w, dtype=np.float32).reshape(FIELD)
    v = np.asarray(v, dtype=np.float32)
    b0 = float(np.asarray(b, dtype=np.float32).reshape(-1)[0])

    s64 = (v.astype(np.float64) ** 2).sum(axis=1)
    sqs = np.sqrt(s64)                      # [FIELD]
    vp = (v / sqs[:, None].astype(np.float32)).astype(np.float16)
    wp = (w / sqs.astype(np.float32)).astype(np.float16)
    vw = np.concatenate([vp, wp[:, None]], axis=1)  # [FIELD, M] fp16

    def pack(a):  # [FIELD, M] -> [128, KTILES*M] SBUF image
        return np.ascontiguousarray(
            a.reshape(KTILES, 128, M).transpose(1, 0, 2).reshape(128, KTILES * M)
        )

    vwi = pack(vw)

    red = np.zeros((M, 1), np.float16)
    red[0:EMBED, 0] = 0.5
    red[EMBED, 0] = 1.0
    onesn = np.full((128, 1), -0.5, np.float16)
    bvec = np.full((1, 1), b0, np.float32)

    u = (x * sqs.astype(np.float32)[None, :]).astype(np.float16)  # [BATCH, FIELD]

    in_maps = []
    for c in range(NCORES):
        ut_c = np.ascontiguousarray(u[c * BS:(c + 1) * BS, :].T)
        in_maps.append({
            "ut": ut_c, "vwi": vwi, "red": red, "onesn": onesn, "bvec": bvec,
        })
    return in_maps


def _run(x, w, b, v, **spmd_kwargs):
    global _NC_CACHE
    if _NC_CACHE is None:
        _NC_CACHE = _build_nc()
    nc = _NC_CACHE

    in_maps = _prep_inputs(x, w, b, v)
    res = run_bass_kernel_spmd(nc, in_maps, list(range(NCORES)), **spmd_kwargs)
    out = np.concatenate(
        [res.results[c]["y"].reshape(BS) for c in range(NCORES)]
    )
    return out.reshape(BATCH, 1).astype(np.float32), res


def kernel(x, w, b, v):
    out, _ = _run(x, w, b, v)
    return out
